# revision 8
# baseline (speedup 1.0000x reference)
"""Trainium2 Bass kernel for nn_Jointer: per-sample masked cosine-similarity.

out[b] = relu(l2norm(source[b]) @ l2norm(target[b]).T) * (mask_src[b] outer mask_tar[b])

The masks kill ~75% of the output (ragged_sequence): only valid source rows x
valid target cols are nonzero. Host side gathers the valid tokens per sample,
l2-normalizes, pre-transposes to [D, tokens] and casts to bf16; the device
computes just the compact relu(sim) block (bf16 in/out, f32 PSUM accumulate);
host scatters the compact block back into the zero-filled full f32 output.
Per core that is ~0.6 MB in + ~2.7 MB out of HBM traffic instead of 18.8 MB
dense f32.

Device: raw bass (no TileContext — its prologue/teardown semaphore walk
costs ~10us). Pipeline per core: load sT/tT on the two HWDGE rings -> per
128-row block m: 3 matmuls into a 3-bank PSUM row tile (2 rotating) -> one
relu+bf16 row drain (ACT even rows, DVE odd) into one of 3 rotating SBUF
row buffers -> row DMA from Sync. All ordering via explicit semaphores.

Sharding: data-parallel over batch B=8 -> one sample per NeuronCore.
"""

import numpy as np
import ml_dtypes

import concourse.bass as bass
from concourse import bacc
import concourse.mybir as mybir
from concourse.bass_utils import run_bass_kernel_spmd

F32 = mybir.dt.float32
BF16 = mybir.dt.bfloat16
AF = mybir.ActivationFunctionType

P = 128  # partitions (= feature dim D = contraction dim)
BANK = 512  # PSUM bank, fp32 elements
EPS = 1e-12


def build_nc(NS, NT) -> bass.Bass:
    nc = bacc.Bacc(trn_type="TRN2")

    sT = nc.dram_tensor("sT", [P, NS], BF16, kind="ExternalInput")
    tT = nc.dram_tensor("tT", [P, NT], BF16, kind="ExternalInput")
    out = nc.dram_tensor("out", [NS, NT], BF16, kind="ExternalOutput")
    out_r = out.rearrange("(m p) n -> m p n", p=P)
    sT_r = sT.rearrange("p n -> p n")
    tT_r = tT.rearrange("p n -> p n")

    MB = NS // P
    ch = []
    pos = 0
    while pos < NT:
        w = min(BANK, NT - pos)
        ch.append((pos, w))
        pos += w
    NB = len(ch)  # PSUM banks per row tile
    NOB = 3  # rotating SBUF output row buffers

    with (
        nc.semaphore("s_in") as s_in,
        nc.semaphore("s_mm") as s_mm,
        nc.semaphore("s_dra") as s_dra,
        nc.semaphore("s_drv") as s_drv,
        nc.semaphore("s_out") as s_out,
        nc.sbuf_tensor([P, NS], BF16) as sT_sb,
        nc.sbuf_tensor([P, NT], BF16) as tT_sb,
        nc.sbuf_tensor([P, NOB * NT], BF16) as ob,
        nc.psum_tensor([P, NB * BANK], F32) as ps0,
        nc.psum_tensor([P, NB * BANK], F32) as ps1,
    ):
        psb = [ps0, ps1]

        def drain_wait(eng, m):
            # all NB matmuls of row m done
            eng.wait_ge(s_mm, NB * (m + 1))
            # ob slot reuse: row m-NOB's DMA completed
            d = m - NOB
            if d >= 0:
                eng.wait_ge(s_out, 16 * (d + 1))

        with nc.Block() as block:

            @block.sync
            def _(sync):
                sync.dma_start(sT_sb[:, :], sT_r[:, :]).then_inc(s_in, 16)
                for m in range(MB):
                    if m % 2 == 0:
                        sync.wait_ge(s_dra, m // 2 + 1)
                    else:
                        sync.wait_ge(s_drv, (m + 1) // 2)
                    sl = (m % NOB) * NT
                    sync.dma_start(out_r[m], ob[:, sl : sl + NT]).then_inc(
                        s_out, 16
                    )
                sync.wait_ge(s_out, 16 * MB)

            @block.scalar
            def _(scalar):
                scalar.dma_start(tT_sb[:, :], tT_r[:, :]).then_inc(s_in, 16)
                for m in range(0, MB, 2):
                    drain_wait(scalar, m)
                    sl = (m % NOB) * NT
                    scalar.activation(
                        out=ob[:, sl : sl + NT],
                        in_=psb[m % 2][:, :NT],
                        func=AF.Relu,
                    ).then_inc(s_dra, 1)

            @block.vector
            def _(vector):
                for m in range(1, MB, 2):
                    drain_wait(vector, m)
                    sl = (m % NOB) * NT
                    vector.tensor_scalar_max(
                        out=ob[:, sl : sl + NT],
                        in0=psb[m % 2][:, :NT],
                        scalar1=0.0,
                    ).then_inc(s_drv, 1)

            @block.tensor
            def _(tensor):
                tensor.wait_ge(s_in, 32)
                for m in range(MB):
                    d = m - 2  # psum row tile reuse: row m-2 drained
                    if d >= 0:
                        if d % 2 == 0:
                            tensor.wait_ge(s_dra, d // 2 + 1)
                        else:
                            tensor.wait_ge(s_drv, (d + 1) // 2)
                    for n0, w in ch:
                        tensor.matmul(
                            psb[m % 2][:, n0 : n0 + w],
                            sT_sb[:, m * P : (m + 1) * P],
                            tT_sb[:, n0 : n0 + w],
                            start=True,
                            stop=True,
                        ).then_inc(s_mm, 1)

    nc.compile()
    return nc


_NC_CACHE = {}


def _get_nc(NS, NT):
    key = (NS, NT)
    if key not in _NC_CACHE:
        _NC_CACHE[key] = build_nc(NS, NT)
    return _NC_CACHE[key]


def _pad128(n):
    return max(P, -(-n // P) * P)


def kernel(source, target, mask_src, mask_tar, **run_kwargs):
    source = np.asarray(source, dtype=np.float32)
    target = np.asarray(target, dtype=np.float32)
    mask_src = np.asarray(mask_src).astype(bool)
    mask_tar = np.asarray(mask_tar).astype(bool)
    B, S, D = source.shape
    T = target.shape[1]

    idx_s = [np.flatnonzero(mask_src[b]) for b in range(B)]
    idx_t = [np.flatnonzero(mask_tar[b]) for b in range(B)]
    NS = _pad128(max(len(i) for i in idx_s))
    NT = _pad128(max(len(i) for i in idx_t))

    in_maps = []
    for b in range(B):
        s = source[b][idx_s[b]]
        t = target[b][idx_t[b]]
        s = s / np.maximum(np.linalg.norm(s, axis=1, keepdims=True), EPS)
        t = t / np.maximum(np.linalg.norm(t, axis=1, keepdims=True), EPS)
        sTb = np.zeros((P, NS), dtype=ml_dtypes.bfloat16)
        tTb = np.zeros((P, NT), dtype=ml_dtypes.bfloat16)
        sTb[:, : len(idx_s[b])] = s.T.astype(ml_dtypes.bfloat16)
        tTb[:, : len(idx_t[b])] = t.T.astype(ml_dtypes.bfloat16)
        in_maps.append({"sT": sTb, "tT": tTb})

    nc = _get_nc(NS, NT)
    res = run_bass_kernel_spmd(nc, in_maps, core_ids=list(range(B)), **run_kwargs)

    full = np.zeros((B, S, T), dtype=np.float32)
    for b in range(B):
        oc = np.asarray(res.results[b]["out"]).astype(np.float32)
        ns, nt = len(idx_s[b]), len(idx_t[b])
        if ns and nt:
            full[b][np.ix_(idx_s[b], idx_t[b])] = oc[:ns, :nt]
    if run_kwargs.get("trace"):
        kernel.last_results = res
    return full


# revision 12
# speedup vs baseline: 1.0678x; 1.0678x over previous
"""Trainium2 Bass kernel for nn_Jointer: per-sample masked cosine-similarity.

out[b] = relu(l2norm(source[b]) @ l2norm(target[b]).T) * (mask_src[b] outer mask_tar[b])

The masks kill ~75% of the output (ragged_sequence): only valid source rows x
valid target cols are nonzero. Host side gathers the valid tokens per sample,
l2-normalizes, pre-transposes to [D, tokens] and casts to bf16; the device
computes just the compact relu(sim) block (bf16 in/out, f32 PSUM accumulate);
host scatters the compact block back into the zero-filled full f32 output.
Per core that is ~0.6 MB in + ~2.7 MB out of HBM traffic instead of 18.8 MB
dense f32.

Device: raw bass (no TileContext — its prologue/teardown semaphore walk costs
~10us). Chunk-granular pipeline per core: chunked input loads on the two
HWDGE rings (mm0 starts as soon as the first chunks' completion sems land);
27 matmuls (384-wide, one PSUM bank each, 6-bank rotation); relu+bf16 chunk
drains alternating ACT/DVE into 3 rotating SBUF row buffers; output DMAs
from Sync (row-level, chunk-level on first/last rows to shorten ramp/tail).
All ordering via explicit semaphores.

Sharding: data-parallel over batch B=8 -> one sample per NeuronCore.
"""

import numpy as np
import ml_dtypes

import concourse.bass as bass
from concourse import bacc
import concourse.mybir as mybir
from concourse.bass_utils import run_bass_kernel_spmd

F32 = mybir.dt.float32
BF16 = mybir.dt.bfloat16
AF = mybir.ActivationFunctionType

P = 128  # partitions (= feature dim D = contraction dim)
EPS = 1e-12


def _chunks(n, cap=512):
    """Split n (multiple of 128) into near-equal multiples of 128, each <= cap."""
    k = -(-n // cap)
    base = n // k // P * P
    rem = (n - base * k) // P
    widths = [base + P if i < rem else base for i in range(k)]
    out, pos = [], 0
    for w in widths:
        out.append((pos, w))
        pos += w
    return out


def build_nc(NS, NT) -> bass.Bass:
    nc = bacc.Bacc(trn_type="TRN2")

    sT = nc.dram_tensor("sT", [P, NS], BF16, kind="ExternalInput")
    tT = nc.dram_tensor("tT", [P, NT], BF16, kind="ExternalInput")
    out = nc.dram_tensor("out", [NS, NT], BF16, kind="ExternalOutput")
    out_r = out.rearrange("(m p) n -> m p n", p=P)
    sT_r = sT.rearrange("p n -> p n")
    tT_r = tT.rearrange("p n -> p n")

    MB = NS // P
    ch = _chunks(NT)
    NCH = len(ch)
    NPS = 2 * NCH  # rotating single-bank PSUM tiles
    NOB = 3  # rotating SBUF output row buffers
    NK = MB * NCH  # total chunks

    # chunk k -> engine parity: even = ACT, odd = DVE
    def n_act(k):  # ACT drains among chunks < k
        return (k + 1) // 2

    def n_dve(k):
        return k // 2

    # out-DMA plan: chunk-level on rows 0 and MB-1, row-level between.
    # DMA completions across separate dma_starts are NOT ordered, so every
    # cross-DMA dependency gets its own semaphore with an exact target:
    #   s_ins0/s_ins1: the two sT load chunks;  s_int[ci]: tT chunk ci
    #   s_ob[slot]: out-DMA completions per rotating ob slot (exact ob reuse)
    # Engine-local completion sems (in-order per engine) stay shared counters:
    #   s_mm (PE), s_dra (ACT), s_drv (DVE).

    def slot_done(m):
        """s_ob[m % NOB] target once rows {r < m, r = m mod NOB} are out."""
        tot = 0
        for r in range(m % NOB, m, NOB):
            tot += 16 * (NCH if r in (0, MB - 1) else 1)
        return tot

    import contextlib

    with contextlib.ExitStack() as stack:
        ec = stack.enter_context
        s_ins0 = ec(nc.semaphore("s_ins0"))
        s_ins1 = ec(nc.semaphore("s_ins1"))
        s_int = [ec(nc.semaphore(f"s_int{i}")) for i in range(NCH)]
        s_mm = ec(nc.semaphore("s_mm"))
        s_dra = ec(nc.semaphore("s_dra"))
        s_drv = ec(nc.semaphore("s_drv"))
        s_ob = [ec(nc.semaphore(f"s_ob{i}")) for i in range(NOB)]
        sT_sb = ec(nc.sbuf_tensor("sT_sb", [P, NS], BF16))
        tT_sb = ec(nc.sbuf_tensor("tT_sb", [P, NT], BF16))
        ob = ec(nc.sbuf_tensor("ob", [P, NOB * NT], BF16))
        psb = [
            ec(nc.psum_tensor(f"psb{i}", [P, ch[0][1]], F32))
            for i in range(NPS)
        ]

        def drain_waits(eng, k, prev):
            """Waits before draining chunk k; prev = [mm, ob0..] thresholds
            already waited on this engine (dedupe monotone waits)."""
            m = k // NCH
            if k + 1 > prev[0]:
                eng.wait_ge(s_mm, k + 1)
                prev[0] = k + 1
            if m >= NOB:  # ob slot reuse
                sl = m % NOB
                tgt = slot_done(m)
                if tgt > prev[1 + sl]:
                    eng.wait_ge(s_ob[sl], tgt)
                    prev[1 + sl] = tgt

        with nc.Block() as block:

            @block.sync
            def _(sync):
                sync.dma_start(sT_sb[:, :P], sT_r[:, :P]).then_inc(s_ins0, 16)
                sync.dma_start(sT_sb[:, P:], sT_r[:, P:]).then_inc(s_ins1, 16)
                pa = pv = 0
                for m in range(MB):
                    sl = (m % NOB) * NT
                    if m in (0, MB - 1):
                        for ci, (n0, w) in enumerate(ch):
                            k = m * NCH + ci
                            if k % 2 == 0 and n_act(k + 1) > pa:
                                pa = n_act(k + 1)
                                sync.wait_ge(s_dra, pa)
                            elif k % 2 == 1 and n_dve(k + 1) > pv:
                                pv = n_dve(k + 1)
                                sync.wait_ge(s_drv, pv)
                            sync.dma_start(
                                out_r[m][:, n0 : n0 + w],
                                ob[:, sl + n0 : sl + n0 + w],
                            ).then_inc(s_ob[m % NOB], 16)
                    else:
                        last = (m + 1) * NCH  # chunks < last all drained
                        if n_act(last) > pa:
                            pa = n_act(last)
                            sync.wait_ge(s_dra, pa)
                        if n_dve(last) > pv:
                            pv = n_dve(last)
                            sync.wait_ge(s_drv, pv)
                        sync.dma_start(
                            out_r[m], ob[:, sl : sl + NT]
                        ).then_inc(s_ob[m % NOB], 16)
                for sl in range(NOB):
                    tot = sum(
                        16 * (NCH if r in (0, MB - 1) else 1)
                        for r in range(sl, MB, NOB)
                    )
                    if tot:
                        sync.wait_ge(s_ob[sl], tot)

            @block.scalar
            def _(scalar):
                for ci, (n0, w) in enumerate(ch):
                    scalar.dma_start(
                        tT_sb[:, n0 : n0 + w], tT_r[:, n0 : n0 + w]
                    ).then_inc(s_int[ci], 16)
                prev = [0] + [0] * NOB
                for k in range(0, NK, 2):
                    m, ci = k // NCH, k % NCH
                    n0, w = ch[ci]
                    drain_waits(scalar, k, prev)
                    sl = (m % NOB) * NT
                    scalar.activation(
                        out=ob[:, sl + n0 : sl + n0 + w],
                        in_=psb[k % NPS][:, :w],
                        func=AF.Relu,
                    ).then_inc(s_dra, 1)

            @block.vector
            def _(vector):
                prev = [0] + [0] * NOB
                for k in range(1, NK, 2):
                    m, ci = k // NCH, k % NCH
                    n0, w = ch[ci]
                    drain_waits(vector, k, prev)
                    sl = (m % NOB) * NT
                    vector.tensor_scalar_max(
                        out=ob[:, sl + n0 : sl + n0 + w],
                        in0=psb[k % NPS][:, :w],
                        scalar1=0.0,
                    ).then_inc(s_drv, 1)

            @block.tensor
            def _(tensor):
                pa = pv = 0
                for k in range(NK):
                    m, ci = k // NCH, k % NCH
                    n0, w = ch[ci]
                    if k == ci:  # first row: tT chunk ci needed
                        tensor.wait_ge(s_int[ci], 16)
                        if k == 0:
                            tensor.wait_ge(s_ins0, 16)
                    if k == NCH:  # second row: rest of sT needed
                        tensor.wait_ge(s_ins1, 16)
                    d = k - NPS  # PSUM slot reuse: chunk k-NPS drained
                    if d >= 0:
                        if d % 2 == 0 and n_act(d + 1) > pa:
                            pa = n_act(d + 1)
                            tensor.wait_ge(s_dra, pa)
                        elif d % 2 == 1 and n_dve(d + 1) > pv:
                            pv = n_dve(d + 1)
                            tensor.wait_ge(s_drv, pv)
                    tensor.matmul(
                        psb[k % NPS][:, :w],
                        sT_sb[:, m * P : (m + 1) * P],
                        tT_sb[:, n0 : n0 + w],
                        start=True,
                        stop=True,
                    ).then_inc(s_mm, 1)

        nc.compile()
    return nc


_NC_CACHE = {}


def _get_nc(NS, NT):
    key = (NS, NT)
    if key not in _NC_CACHE:
        _NC_CACHE[key] = build_nc(NS, NT)
    return _NC_CACHE[key]


def _pad128(n):
    return max(P, -(-n // P) * P)


def kernel(source, target, mask_src, mask_tar, **run_kwargs):
    source = np.asarray(source, dtype=np.float32)
    target = np.asarray(target, dtype=np.float32)
    mask_src = np.asarray(mask_src).astype(bool)
    mask_tar = np.asarray(mask_tar).astype(bool)
    B, S, D = source.shape
    T = target.shape[1]

    idx_s = [np.flatnonzero(mask_src[b]) for b in range(B)]
    idx_t = [np.flatnonzero(mask_tar[b]) for b in range(B)]
    NS = _pad128(max(len(i) for i in idx_s))
    NT = _pad128(max(len(i) for i in idx_t))

    in_maps = []
    for b in range(B):
        s = source[b][idx_s[b]]
        t = target[b][idx_t[b]]
        s = s / np.maximum(np.linalg.norm(s, axis=1, keepdims=True), EPS)
        t = t / np.maximum(np.linalg.norm(t, axis=1, keepdims=True), EPS)
        sTb = np.zeros((P, NS), dtype=ml_dtypes.bfloat16)
        tTb = np.zeros((P, NT), dtype=ml_dtypes.bfloat16)
        sTb[:, : len(idx_s[b])] = s.T.astype(ml_dtypes.bfloat16)
        tTb[:, : len(idx_t[b])] = t.T.astype(ml_dtypes.bfloat16)
        in_maps.append({"sT": sTb, "tT": tTb})

    nc = _get_nc(NS, NT)
    res = run_bass_kernel_spmd(nc, in_maps, core_ids=list(range(B)), **run_kwargs)

    full = np.zeros((B, S, T), dtype=np.float32)
    for b in range(B):
        oc = np.asarray(res.results[b]["out"]).astype(np.float32)
        ns, nt = len(idx_s[b]), len(idx_t[b])
        if ns and nt:
            full[b][np.ix_(idx_s[b], idx_t[b])] = oc[:ns, :nt]
    if run_kwargs.get("trace"):
        kernel.last_results = res
    return full
